# revision 23
# baseline (speedup 1.0000x reference)
"""Trainium2 Bass kernel for GNO message passing (nn_GNO_69312182222948).

Strategy (data-parallel over edges, 8 cores):
  - Host gathers per-edge rel = x_sparse[src] - x_dense[dst] (numpy fancy
    indexing) in bf16 and packs it partition-dense: 4 chunks x 30 rows
    (10 streams x 3 dims) at 32-aligned partition bases, 2048 cols per
    input tile -> [128, 2048] bf16 DMA tiles (4 KB per partition line).
  - Device: per-edge kernel MLP 3->12->12->3 on TensorE in bf16 (full
    rate; fp32 runs a 2x LOW_HIGH decomposition and fp16 streams at half
    rate). Layer-1 contracts K=30 per chunk (explicit 32-aligned
    tile_position row bases); layer-2 K=120 block-diagonal; layer-3
    accumulates 4 units of k into one [128, 512] PSUM tile at 32-row
    offsets via block weight variants so evacuation runs at full
    partition width.
  - GELU is split across two engines: layer-1 GELU runs on VectorE via a
    custom fused DVE op (single-instruction polynomial
    2*gelu(x) ~= x + x^2*(e0 + e1 x^2 + e2 x^4), coefficients minimax-fit
    at runtime for the exact |a1| range implied by W1; W2 is pre-halved on
    the host to absorb the factor 2). Layer-2 GELU runs on ScalarE's
    exact table. k evacuation copies alternate ScalarE/VectorE.
  - A ~7 us warm-up matmul burst plus first-tile gap-filler matmuls keep
    the PE HAM clock-gate at 8/8 (2.4 GHz) through pipeline ramp-up;
    without them the PE idles past the activity window and runs the
    first ~70 us at 1.2 GHz.
  - Device streams k back in bf16; host applies (k + b3) * f_sparse[src],
    the sorted segment mean (np.add.reduceat) and the tiny projection MLP.
"""

import numpy as np
import ml_dtypes

BF16 = ml_dtypes.bfloat16

import concourse.bass as bass
import concourse.mybir as mybir
from concourse.bacc import Bacc
from concourse.tile import TileContext
from concourse.bass_utils import run_bass_kernel_spmd

# Problem sizes (hardcoded per contract)
N_S = 131072
N_D = 131072
E = 8388608
DIM = 3
H = 12

N_CORES = 8
S = 10                      # streams (10 * 12 = 120 hidden partitions)
TW = 2048                   # cols per input tile per chunk
NCHUNK = 4                  # chunks per input tile (row bases 0/32/64/96)
NT = 13                     # input tiles per core
C_PC = NT * NCHUNK * TW     # edge-columns per core = 106496
E_PC = S * C_PC             # edges per core (padded) = 1064960
E_PAD = N_CORES * E_PC      # total padded edges = 8519680

# weight table columns (fp16): [0:120]=w1 (4 row-base variants share cols),
# [120:240]=w2 (block-diag, pre-halved when DVE gelu1), [240:240+4*128]=w3
# accumulate variants
W1C, W2C, W3C = 0, 120, 240
WCOLS = 240 + 4 * 128

_BASS_CACHE = {}
_GELU_OP = None


def _register_gelu_op():
    """Register the fused polynomial-GELU custom DVE op (idempotent)."""
    global _GELU_OP
    if _GELU_OP is not None:
        return _GELU_OP
    from concourse import dve_ops as dops
    from concourse.dve_spec import Spec, Src0, C0, C1, C2, sq, lower
    from concourse.dve_uop import DveOpSpec

    name = "GELU2X_POLY_ANT"
    if name in dops._SUB_OPCODE_FOR_NAME:
        _GELU_OP = next(op for op in dops.OPS if op.name == name)
        return _GELU_OP

    u = sq(Src0)
    r = (u * C2 + C1) * u + C0
    spec = Spec(
        body=u * r + Src0,
        reference=lambda in0, in1, s0, s1, imm2: (
            (in0.astype(np.float32) ** 2)
            * (((in0.astype(np.float32) ** 2) * imm2 + s1)
               * (in0.astype(np.float32) ** 2) + s0)
            + in0.astype(np.float32)
        ),
    )
    row = dops._CUSTOM_DVE_ROW_BASE + len(dops.OPS)
    shas = {}
    for ver in ("v3", "v4"):
        uops = lower(spec, ver=ver)
        shas[ver] = DveOpSpec(name=name, opcode=row, uops=uops,
                              rd1_en=False).sha(ver)
    op = dops.DveOp(name, spec, subdim=False, uops_sha=shas)
    dops.OPS.append(op)
    dops.CUSTOM_DVE_SPECS[name] = spec
    dops._SUB_OPCODE_FOR_NAME[name] = row
    _GELU_OP = op
    return op


def _fit_gelu_poly(rmax):
    """Minimax-ish fit of x*erf(x/sqrt2) ~= u*(e0 + e1 u + e2 u^2), u=x^2,
    over |x| <= rmax, so that x + fit(x) == 2*gelu(x)."""
    from scipy.special import erf as _erf_fn
    x = np.linspace(1e-6, max(rmax, 0.25), 2001)
    u = x * x
    y = x * _erf_fn(x / np.sqrt(2.0))
    A = np.stack([u, u * u, u * u * u], axis=1)
    w = np.ones_like(x)
    best = None
    for _ in range(120):
        c, *_ = np.linalg.lstsq(A * w[:, None], (y * w)[:, None], rcond=None)
        c = c[:, 0]
        err = A @ c - y
        m = np.abs(err).max()
        if best is None or m < best[1]:
            best = (c, m)
        w = w * (0.9 + 0.25 * np.abs(err) / m)
        w /= w.max()
    return best  # (coeffs, max_abs_err_of_2gelu)


def _build_bass(gelu1_coefs):
    """gelu1_coefs: tuple (e0, e1, e2) for the DVE poly, or None to run
    layer-1 GELU on ScalarE (exact, supports bias b1)."""
    key = gelu1_coefs
    if key in _BASS_CACHE:
        return _BASS_CACHE[key]
    fp32 = mybir.dt.float32
    fp16 = mybir.dt.bfloat16
    GELU = mybir.ActivationFunctionType.Gelu
    use_dve = gelu1_coefs is not None
    if use_dve:
        gop = _register_gelu_op()
        e0, e1, e2 = gelu1_coefs

    nc = Bacc()
    xin = nc.dram_tensor("xin", [NT, 128, TW], fp16, kind="ExternalInput")
    wtab = nc.dram_tensor("wtab", [128, WCOLS], fp16, kind="ExternalInput")
    btab = nc.dram_tensor("btab", [128, 2], fp32, kind="ExternalInput")
    kout = nc.dram_tensor("kout", [NT, 128, TW], fp16, kind="ExternalOutput")

    with TileContext(nc) as tc:
        with (
            tc.tile_pool(name="wpool", bufs=1) as wpool,
            tc.tile_pool(name="inpool", bufs=3) as inpool,
            tc.tile_pool(name="h1gpool", bufs=3) as h1gpool,
            tc.tile_pool(name="h2gpool", bufs=4) as h2gpool,
            tc.tile_pool(name="kspool", bufs=2) as kspool,
            tc.tile_pool(name="ph1", bufs=2, space="PSUM") as ph1,
            tc.tile_pool(name="ph2", bufs=3, space="PSUM") as ph2,
            tc.tile_pool(name="pk", bufs=1, space="PSUM") as pk,
        ):
            wt = wpool.tile([128, WCOLS], fp16, tag="wt")
            nc.sync.dma_start(wt[:], wtab[:, :])
            bt = wpool.tile([128, 2], fp32, tag="bt")
            nc.sync.dma_start(bt[:], btab[:, :])
            w1v = [wt[32 * c:32 * c + 30, W1C:W1C + 120] for c in range(NCHUNK)]
            w2s = wt[0:120, W2C:W2C + 120]
            w3v = [wt[0:120, W3C + 128 * n:W3C + 128 * (n + 1)]
                   for n in range(4)]
            b2t = bt[0:120, 0:1]
            b1t = bt[0:120, 1:2]

            # Warm-up burst: ~8.5 us of back-to-back matmuls trips the PE
            # HAM clock-gate to 8/8 (2.4 GHz) before the real work; without
            # it the ramp-up phase idles long enough that the PE stays at
            # 4/8 (1.2 GHz) for the first ~70 us.
            wup = ph1.tile([120, 1024], fp32, tag="h1", name="wup")
            for r in range(32):
                nc.tensor.matmul(wup[:, 256 * (r % 4):256 * (r % 4) + 256],
                                 w2s, wt[0:120, 0:256],
                                 start=True, stop=True)

            for t in range(NT):
                xt = inpool.tile([128, TW], fp16, tag="x")
                nc.sync.dma_start(xt[:], xin[t, :, :])
                ks = kspool.tile([128, TW], fp16, tag="ks")
                for c in range(NCHUNK):
                    ramp = t == 0
                    ka = pk.tile([128, 512], fp32, tag="ka")
                    for o in range(2):
                        h1 = ph1.tile([120, 1024], fp32, tag="h1")
                        xo = 1024 * o
                        for q in range(2):
                            nc.tensor.matmul(
                                h1[:, 512 * q:512 * q + 512], w1v[c],
                                xt[32 * c:32 * c + 30,
                                   xo + 512 * q:xo + 512 * q + 512],
                                start=True, stop=True,
                                tile_position=(32 * c, 0))
                        h1g = h1gpool.tile([120, 1024], fp16, tag="h1g")
                        if use_dve:
                            nc.vector._custom_dve(
                                gop, out=h1g[:], in0=h1[:],
                                s0=float(e0), s1=float(e1), imm2=float(e2))
                        else:
                            nc.scalar.activation(h1g[:], h1[:], GELU,
                                                 bias=b1t)
                        for q in range(2):
                            n = 2 * o + q
                            h2 = ph2.tile([120, 512], fp32, tag="h2")
                            nc.tensor.matmul(h2[:], w2s,
                                             h1g[:, 512 * q:512 * q + 512],
                                             start=True, stop=True)
                            h2g = h2gpool.tile([120, 512], fp16, tag="h2g")
                            nc.scalar.activation(h2g[:], h2[:], GELU,
                                                 bias=b2t)
                            if ramp:
                                # gap filler into the already-consumed h1
                                # PSUM tile (overwritten by L1 next round)
                                nc.tensor.matmul(h1[:, 0:256], w2s,
                                                 wt[0:120, 0:256],
                                                 start=True, stop=True)
                            nc.tensor.matmul(ka[:], w3v[n], h2g[:],
                                             start=(n == 0), stop=(n == 3))
                    # k copies 3:1 VectorE:ScalarE — keeps ScalarE's gelu2
                    # stream nearly pure (layer-3 rarely queues behind a
                    # copy) while evening the two elementwise queues.
                    if (t * NCHUNK + c) % 4 == 3:
                        nc.scalar.copy(ks[:, 512 * c:512 * c + 512], ka[:])
                    else:
                        nc.vector.tensor_copy(ks[:, 512 * c:512 * c + 512],
                                              ka[:])
                nc.gpsimd.dma_start(kout[t, :, :], ks[:])

    nc.finalize()
    _BASS_CACHE[key] = nc
    return nc


def _erf(x):
    # Abramowitz & Stegun 7.1.26 fallback (|err| <= 1.5e-7)
    a1, a2, a3, a4, a5 = (0.254829592, -0.284496736, 1.421413741,
                          -1.453152027, 1.061405429)
    p = 0.3275911
    s = np.sign(x)
    ax = np.abs(x)
    t = 1.0 / (1.0 + p * ax)
    y = 1.0 - (((((a5 * t + a4) * t) + a3) * t + a2) * t + a1) * t * np.exp(-ax * ax)
    return s * y

try:
    from scipy.special import erf as _erf  # noqa: F811
except Exception:
    pass


def _gelu_np(x):
    return 0.5 * x * (1.0 + _erf(x / np.sqrt(2.0)))


def _plan(W1, b1):
    """Pick the gelu1 implementation: DVE poly (needs b1 == 0) with coeffs
    fit to the provable |a1| bound, else exact ScalarE."""
    if np.any(np.asarray(b1) != 0):
        return None
    r1 = 0.5 * np.abs(np.asarray(W1, np.float64)).sum(axis=0).max()
    r1 = float(r1) * 1.02 + 0.02
    coefs, maxerr = _fit_gelu_poly(r1)
    if maxerr > 2.5e-3:  # 2*gelu error budget; fall back to exact
        return None
    return tuple(round(float(v), 10) for v in coefs)


def _pack_inputs(x_sparse, f_sparse, x_dense, W1, b1, W2, b2, W3, b3,
                 edge_src, edge_dst, gelu1_coefs):
    src = np.asarray(edge_src).astype(np.int64)
    dst = np.asarray(edge_dst).astype(np.int64)
    x_sparse = np.asarray(x_sparse, dtype=np.float32)
    x_dense = np.asarray(x_dense, dtype=np.float32)

    rel = np.zeros((E_PAD, DIM), BF16)
    rel[:E] = (x_sparse[src] - x_dense[dst]).astype(BF16)

    W1 = np.asarray(W1, np.float32)
    W2 = np.asarray(W2, np.float32)
    W3 = np.asarray(W3, np.float32)
    if gelu1_coefs is not None:
        W2 = W2 * 0.5  # absorb the DVE op's 2*gelu scale

    wtab = np.zeros((128, WCOLS), BF16)
    rs = np.arange(S)
    for c in range(NCHUNK):
        for j in range(DIM):
            wtab[(32 * c + 3 * rs + j)[:, None],
                 W1C + 12 * rs[:, None] + np.arange(H)] \
                = W1[j].astype(BF16)
    for i in range(H):
        wtab[(12 * rs + i)[:, None], W2C + 12 * rs[:, None] + np.arange(H)] \
            = W2[i].astype(BF16)
    for m in range(4):
        for i in range(H):
            wtab[(12 * rs + i)[:, None], W3C + 128 * m + 32 * m
                 + 3 * rs[:, None] + np.arange(DIM)] = W3[i].astype(BF16)
    btab = np.zeros((128, 2), np.float32)
    btab[12 * rs[:, None] + np.arange(H), 0] = np.asarray(b2, np.float32)
    btab[12 * rs[:, None] + np.arange(H), 1] = np.asarray(b1, np.float32)

    in_maps = []
    for cr in range(N_CORES):
        relc = rel[cr * E_PC:(cr + 1) * E_PC]
        # [S, NT, NCHUNK, TW, DIM] -> [NT, NCHUNK, S, DIM, TW]
        x5 = relc.reshape(S, NT, NCHUNK, TW, DIM).transpose(1, 2, 0, 4, 3)
        x4 = np.zeros((NT, NCHUNK, 32, TW), BF16)
        x4[:, :, :30, :] = x5.reshape(NT, NCHUNK, 30, TW)
        in_maps.append({
            "xin": x4.reshape(NT, 128, TW),
            "wtab": wtab,
            "btab": btab,
        })
    return in_maps, src, dst


def _host_tail(outs, src, dst, f_sparse, b3, P1w, P1b, P2w, P2b, P3w, P3b):
    f_sparse = np.asarray(f_sparse, np.float32)
    b3 = np.asarray(b3, np.float32)
    k = np.empty((E_PAD, DIM), np.float32)
    for cr in range(N_CORES):
        ko = np.asarray(outs[cr]["kout"])  # [NT, 128, TW] fp16
        # rows: 32n + 3s + j; cols: 512*c + v
        k6 = ko.reshape(NT, 4, 32, NCHUNK, 512)[:, :, :30, :, :]
        k6 = k6.reshape(NT, 4, S, DIM, NCHUNK, 512)
        # [t, n, s, j, c, v] -> [s, t, c, n, v, j]
        k6 = k6.transpose(2, 0, 4, 1, 5, 3)
        k[cr * E_PC:(cr + 1) * E_PC] = k6.reshape(E_PC, DIM).astype(np.float32)
    k = k[:E]

    msg = (k + b3) * f_sparse[src]

    cnt = np.bincount(dst, minlength=N_D).astype(np.float32)
    starts = (np.cumsum(cnt) - cnt).astype(np.int64)
    nz = cnt > 0
    sums = np.zeros((N_D, DIM), np.float32)
    if nz.any():
        sums[nz] = np.add.reduceat(msg, starts[nz], axis=0)
    out_feat = sums / np.maximum(cnt, 1.0)[:, None]

    h = _gelu_np(out_feat.astype(np.float64) @ np.asarray(P1w, np.float64)
                 + np.asarray(P1b, np.float64))
    h = _gelu_np(h @ np.asarray(P2w, np.float64) + np.asarray(P2b, np.float64))
    out = h @ np.asarray(P3w, np.float64) + np.asarray(P3b, np.float64)
    return out.astype(np.float32)


def kernel(x_sparse, f_sparse, x_dense, W1, b1, W2, b2, W3, b3,
           P1w, P1b, P2w, P2b, P3w, P3b, edge_src, edge_dst):
    gelu1_coefs = _plan(W1, b1)
    in_maps, src, dst = _pack_inputs(x_sparse, f_sparse, x_dense, W1, b1,
                                     W2, b2, W3, b3, edge_src, edge_dst,
                                     gelu1_coefs)
    nc = _build_bass(gelu1_coefs)
    res = run_bass_kernel_spmd(nc, in_maps, list(range(N_CORES)))
    return _host_tail(res.results, src, dst, f_sparse, b3,
                      P1w, P1b, P2w, P2b, P3w, P3b)


def run_profiled(inputs, tmpdir=None):
    """Run once with tracing enabled; returns BassKernelResults."""
    kw = {k: inputs[k] for k in ("x_sparse", "f_sparse", "x_dense", "W1",
                                 "b1", "W2", "b2", "W3", "b3",
                                 "edge_src", "edge_dst")}
    gelu1_coefs = _plan(kw["W1"], kw["b1"])
    in_maps, _, _ = _pack_inputs(**kw, gelu1_coefs=gelu1_coefs)
    nc = _build_bass(gelu1_coefs)
    return run_bass_kernel_spmd(nc, in_maps, list(range(N_CORES)),
                                trace=True, tmpdir=tmpdir)


# revision 25
# speedup vs baseline: 1.0551x; 1.0551x over previous
"""Trainium2 Bass kernel for GNO message passing (nn_GNO_69312182222948).

Strategy (data-parallel over edges, 8 cores):
  - Host gathers per-edge rel = x_sparse[src] - x_dense[dst] (numpy fancy
    indexing) in bf16 and packs it partition-dense: 4 chunks x 30 rows
    (10 streams x 3 dims) at 32-aligned partition bases, 2048 cols per
    input tile -> [128, 2048] bf16 DMA tiles (4 KB per partition line).
  - Device: per-edge kernel MLP 3->12->12->3 on TensorE in bf16 (full
    rate; fp32 runs a 2x LOW_HIGH decomposition and fp16 streams at half
    rate). Layer-1 contracts K=30 per chunk (explicit 32-aligned
    tile_position row bases); layer-2 K=120 block-diagonal; layer-3
    accumulates 4 units of k into one [128, 512] PSUM tile at 32-row
    offsets via block weight variants so evacuation runs at full
    partition width.
  - GELU is split across two engines: layer-1 GELU runs on VectorE via a
    custom fused DVE op (single-instruction polynomial
    2*gelu(x) ~= x + x^2*(e0 + e1 x^2 + e2 x^4), coefficients minimax-fit
    at runtime for the exact |a1| range implied by W1; W2 is pre-halved on
    the host to absorb the factor 2). Layer-2 GELU runs on ScalarE's
    exact table. k evacuation copies alternate ScalarE/VectorE.
  - A ~7 us warm-up matmul burst plus first-tile gap-filler matmuls keep
    the PE HAM clock-gate at 8/8 (2.4 GHz) through pipeline ramp-up;
    without them the PE idles past the activity window and runs the
    first ~70 us at 1.2 GHz.
  - Device streams k back in bf16; host applies (k + b3) * f_sparse[src],
    the sorted segment mean (np.add.reduceat) and the tiny projection MLP.
"""

import numpy as np
import ml_dtypes

BF16 = ml_dtypes.bfloat16

import concourse.bass as bass
import concourse.mybir as mybir
from concourse.bacc import Bacc
from concourse.tile import TileContext
from concourse.bass_utils import run_bass_kernel_spmd

# Problem sizes (hardcoded per contract)
N_S = 131072
N_D = 131072
E = 8388608
DIM = 3
H = 12

N_CORES = 8
S = 10                      # streams (10 * 12 = 120 hidden partitions)
TW = 2048                   # cols per input tile per chunk
NCHUNK = 4                  # chunks per input tile (row bases 0/32/64/96)
NT = 13                     # input tiles per core
C_PC = NT * NCHUNK * TW     # edge-columns per core = 106496
E_PC = S * C_PC             # edges per core (padded) = 1064960
E_PAD = N_CORES * E_PC      # total padded edges = 8519680

# weight table columns (fp16): [0:120]=w1 (4 row-base variants share cols),
# [120:240]=w2 (block-diag, pre-halved when DVE gelu1), [240:240+4*128]=w3
# accumulate variants
W1C, W2C, W3C = 0, 120, 240
WCOLS = 240 + 4 * 128

_BASS_CACHE = {}
_GELU_OP = None


def _register_gelu_op():
    """Register the fused polynomial-GELU custom DVE op (idempotent)."""
    global _GELU_OP
    if _GELU_OP is not None:
        return _GELU_OP
    from concourse import dve_ops as dops
    from concourse.dve_spec import Spec, Src0, C0, C1, C2, sq, lower
    from concourse.dve_uop import DveOpSpec

    name = "GELU2X_POLY_ANT"
    if name in dops._SUB_OPCODE_FOR_NAME:
        _GELU_OP = next(op for op in dops.OPS if op.name == name)
        return _GELU_OP

    u = sq(Src0)
    r = (u * C2 + C1) * u + C0
    spec = Spec(
        body=u * r + Src0,
        reference=lambda in0, in1, s0, s1, imm2: (
            (in0.astype(np.float32) ** 2)
            * (((in0.astype(np.float32) ** 2) * imm2 + s1)
               * (in0.astype(np.float32) ** 2) + s0)
            + in0.astype(np.float32)
        ),
    )
    row = dops._CUSTOM_DVE_ROW_BASE + len(dops.OPS)
    shas = {}
    for ver in ("v3", "v4"):
        uops = lower(spec, ver=ver)
        shas[ver] = DveOpSpec(name=name, opcode=row, uops=uops,
                              rd1_en=False).sha(ver)
    op = dops.DveOp(name, spec, subdim=False, uops_sha=shas)
    dops.OPS.append(op)
    dops.CUSTOM_DVE_SPECS[name] = spec
    dops._SUB_OPCODE_FOR_NAME[name] = row
    _GELU_OP = op
    return op


def _fit_gelu_poly(rmax):
    """Minimax-ish fit of x*erf(x/sqrt2) ~= u*(e0 + e1 u + e2 u^2), u=x^2,
    over |x| <= rmax, so that x + fit(x) == 2*gelu(x)."""
    from scipy.special import erf as _erf_fn
    x = np.linspace(1e-6, max(rmax, 0.25), 2001)
    u = x * x
    y = x * _erf_fn(x / np.sqrt(2.0))
    A = np.stack([u, u * u, u * u * u], axis=1)
    w = np.ones_like(x)
    best = None
    for _ in range(120):
        c, *_ = np.linalg.lstsq(A * w[:, None], (y * w)[:, None], rcond=None)
        c = c[:, 0]
        err = A @ c - y
        m = np.abs(err).max()
        if best is None or m < best[1]:
            best = (c, m)
        w = w * (0.9 + 0.25 * np.abs(err) / m)
        w /= w.max()
    return best  # (coeffs, max_abs_err_of_2gelu)


def _build_bass(gelu1_coefs):
    """gelu1_coefs: tuple (e0, e1, e2) for the DVE poly, or None to run
    layer-1 GELU on ScalarE (exact, supports bias b1)."""
    key = gelu1_coefs
    if key in _BASS_CACHE:
        return _BASS_CACHE[key]
    fp32 = mybir.dt.float32
    fp16 = mybir.dt.bfloat16
    GELU = mybir.ActivationFunctionType.Gelu
    use_dve = gelu1_coefs is not None
    if use_dve:
        gop = _register_gelu_op()
        e0, e1, e2 = gelu1_coefs

    nc = Bacc()
    xin = nc.dram_tensor("xin", [NT, 128, TW], fp16, kind="ExternalInput")
    wtab = nc.dram_tensor("wtab", [128, WCOLS], fp16, kind="ExternalInput")
    btab = nc.dram_tensor("btab", [128, 2], fp32, kind="ExternalInput")
    kout = nc.dram_tensor("kout", [NT, 128, TW], fp16, kind="ExternalOutput")

    with TileContext(nc) as tc:
        with (
            tc.tile_pool(name="wpool", bufs=1) as wpool,
            tc.tile_pool(name="inpool", bufs=5) as inpool,
            tc.tile_pool(name="h1gpool", bufs=4) as h1gpool,
            tc.tile_pool(name="h2gpool", bufs=6) as h2gpool,
            tc.tile_pool(name="kspool", bufs=2) as kspool,
            tc.tile_pool(name="ph1", bufs=2, space="PSUM") as ph1,
            tc.tile_pool(name="ph2", bufs=3, space="PSUM") as ph2,
            tc.tile_pool(name="pk", bufs=1, space="PSUM") as pk,
        ):
            wt = wpool.tile([128, WCOLS], fp16, tag="wt")
            nc.sync.dma_start(wt[:], wtab[:, :])
            bt = wpool.tile([128, 2], fp32, tag="bt")
            nc.sync.dma_start(bt[:], btab[:, :])
            w1v = [wt[32 * c:32 * c + 30, W1C:W1C + 120] for c in range(NCHUNK)]
            w2s = wt[0:120, W2C:W2C + 120]
            w3v = [wt[0:120, W3C + 128 * n:W3C + 128 * (n + 1)]
                   for n in range(4)]
            b2t = bt[0:120, 0:1]
            b1t = bt[0:120, 1:2]

            # Warm-up burst: ~8.5 us of back-to-back matmuls trips the PE
            # HAM clock-gate to 8/8 (2.4 GHz) before the real work; without
            # it the ramp-up phase idles long enough that the PE stays at
            # 4/8 (1.2 GHz) for the first ~70 us.
            wup = ph1.tile([120, 1024], fp32, tag="h1", name="wup")
            for r in range(32):
                nc.tensor.matmul(wup[:, 256 * (r % 4):256 * (r % 4) + 256],
                                 w2s, wt[0:120, 0:256],
                                 start=True, stop=True)

            for t in range(NT):
                xt = inpool.tile([128, TW], fp16, tag="x")
                nc.sync.dma_start(xt[:], xin[t, :, :])
                ks = kspool.tile([128, TW], fp16, tag="ks")
                for c in range(NCHUNK):
                    ramp = t == 0
                    ka = pk.tile([128, 512], fp32, tag="ka")
                    for o in range(2):
                        h1 = ph1.tile([120, 1024], fp32, tag="h1")
                        xo = 1024 * o
                        for q in range(2):
                            nc.tensor.matmul(
                                h1[:, 512 * q:512 * q + 512], w1v[c],
                                xt[32 * c:32 * c + 30,
                                   xo + 512 * q:xo + 512 * q + 512],
                                start=True, stop=True,
                                tile_position=(32 * c, 0))
                        h1g = h1gpool.tile([120, 1024], fp16, tag="h1g")
                        if use_dve:
                            nc.vector._custom_dve(
                                gop, out=h1g[:], in0=h1[:],
                                s0=float(e0), s1=float(e1), imm2=float(e2))
                        else:
                            nc.scalar.activation(h1g[:], h1[:], GELU,
                                                 bias=b1t)
                        for q in range(2):
                            n = 2 * o + q
                            h2 = ph2.tile([120, 512], fp32, tag="h2")
                            nc.tensor.matmul(h2[:], w2s,
                                             h1g[:, 512 * q:512 * q + 512],
                                             start=True, stop=True)
                            h2g = h2gpool.tile([120, 512], fp16, tag="h2g")
                            nc.scalar.activation(h2g[:], h2[:], GELU,
                                                 bias=b2t)
                            if ramp:
                                # gap filler into the already-consumed h1
                                # PSUM tile (overwritten by L1 next round)
                                nc.tensor.matmul(h1[:, 0:256], w2s,
                                                 wt[0:120, 0:256],
                                                 start=True, stop=True)
                            nc.tensor.matmul(ka[:], w3v[n], h2g[:],
                                             start=(n == 0), stop=(n == 3))
                    # All k copies on VectorE: keeps ScalarE a pure gelu2
                    # stream so layer-3 never queues behind a copy.
                    nc.vector.tensor_copy(ks[:, 512 * c:512 * c + 512],
                                          ka[:])
                nc.gpsimd.dma_start(kout[t, :, :], ks[:])

    nc.finalize()
    _BASS_CACHE[key] = nc
    return nc


def _erf(x):
    # Abramowitz & Stegun 7.1.26 fallback (|err| <= 1.5e-7)
    a1, a2, a3, a4, a5 = (0.254829592, -0.284496736, 1.421413741,
                          -1.453152027, 1.061405429)
    p = 0.3275911
    s = np.sign(x)
    ax = np.abs(x)
    t = 1.0 / (1.0 + p * ax)
    y = 1.0 - (((((a5 * t + a4) * t) + a3) * t + a2) * t + a1) * t * np.exp(-ax * ax)
    return s * y

try:
    from scipy.special import erf as _erf  # noqa: F811
except Exception:
    pass


def _gelu_np(x):
    return 0.5 * x * (1.0 + _erf(x / np.sqrt(2.0)))


def _plan(W1, b1):
    """Pick the gelu1 implementation: DVE poly (needs b1 == 0) with coeffs
    fit to the provable |a1| bound, else exact ScalarE."""
    if np.any(np.asarray(b1) != 0):
        return None
    r1 = 0.5 * np.abs(np.asarray(W1, np.float64)).sum(axis=0).max()
    r1 = float(r1) * 1.02 + 0.02
    coefs, maxerr = _fit_gelu_poly(r1)
    if maxerr > 2.5e-3:  # 2*gelu error budget; fall back to exact
        return None
    return tuple(round(float(v), 10) for v in coefs)


def _pack_inputs(x_sparse, f_sparse, x_dense, W1, b1, W2, b2, W3, b3,
                 edge_src, edge_dst, gelu1_coefs):
    src = np.asarray(edge_src).astype(np.int64)
    dst = np.asarray(edge_dst).astype(np.int64)
    x_sparse = np.asarray(x_sparse, dtype=np.float32)
    x_dense = np.asarray(x_dense, dtype=np.float32)

    rel = np.zeros((E_PAD, DIM), BF16)
    rel[:E] = (x_sparse[src] - x_dense[dst]).astype(BF16)

    W1 = np.asarray(W1, np.float32)
    W2 = np.asarray(W2, np.float32)
    W3 = np.asarray(W3, np.float32)
    if gelu1_coefs is not None:
        W2 = W2 * 0.5  # absorb the DVE op's 2*gelu scale

    wtab = np.zeros((128, WCOLS), BF16)
    rs = np.arange(S)
    for c in range(NCHUNK):
        for j in range(DIM):
            wtab[(32 * c + 3 * rs + j)[:, None],
                 W1C + 12 * rs[:, None] + np.arange(H)] \
                = W1[j].astype(BF16)
    for i in range(H):
        wtab[(12 * rs + i)[:, None], W2C + 12 * rs[:, None] + np.arange(H)] \
            = W2[i].astype(BF16)
    for m in range(4):
        for i in range(H):
            wtab[(12 * rs + i)[:, None], W3C + 128 * m + 32 * m
                 + 3 * rs[:, None] + np.arange(DIM)] = W3[i].astype(BF16)
    btab = np.zeros((128, 2), np.float32)
    btab[12 * rs[:, None] + np.arange(H), 0] = np.asarray(b2, np.float32)
    btab[12 * rs[:, None] + np.arange(H), 1] = np.asarray(b1, np.float32)

    in_maps = []
    for cr in range(N_CORES):
        relc = rel[cr * E_PC:(cr + 1) * E_PC]
        # [S, NT, NCHUNK, TW, DIM] -> [NT, NCHUNK, S, DIM, TW]
        x5 = relc.reshape(S, NT, NCHUNK, TW, DIM).transpose(1, 2, 0, 4, 3)
        x4 = np.zeros((NT, NCHUNK, 32, TW), BF16)
        x4[:, :, :30, :] = x5.reshape(NT, NCHUNK, 30, TW)
        in_maps.append({
            "xin": x4.reshape(NT, 128, TW),
            "wtab": wtab,
            "btab": btab,
        })
    return in_maps, src, dst


def _host_tail(outs, src, dst, f_sparse, b3, P1w, P1b, P2w, P2b, P3w, P3b):
    f_sparse = np.asarray(f_sparse, np.float32)
    b3 = np.asarray(b3, np.float32)
    k = np.empty((E_PAD, DIM), np.float32)
    for cr in range(N_CORES):
        ko = np.asarray(outs[cr]["kout"])  # [NT, 128, TW] fp16
        # rows: 32n + 3s + j; cols: 512*c + v
        k6 = ko.reshape(NT, 4, 32, NCHUNK, 512)[:, :, :30, :, :]
        k6 = k6.reshape(NT, 4, S, DIM, NCHUNK, 512)
        # [t, n, s, j, c, v] -> [s, t, c, n, v, j]
        k6 = k6.transpose(2, 0, 4, 1, 5, 3)
        k[cr * E_PC:(cr + 1) * E_PC] = k6.reshape(E_PC, DIM).astype(np.float32)
    k = k[:E]

    msg = (k + b3) * f_sparse[src]

    cnt = np.bincount(dst, minlength=N_D).astype(np.float32)
    starts = (np.cumsum(cnt) - cnt).astype(np.int64)
    nz = cnt > 0
    sums = np.zeros((N_D, DIM), np.float32)
    if nz.any():
        sums[nz] = np.add.reduceat(msg, starts[nz], axis=0)
    out_feat = sums / np.maximum(cnt, 1.0)[:, None]

    h = _gelu_np(out_feat.astype(np.float64) @ np.asarray(P1w, np.float64)
                 + np.asarray(P1b, np.float64))
    h = _gelu_np(h @ np.asarray(P2w, np.float64) + np.asarray(P2b, np.float64))
    out = h @ np.asarray(P3w, np.float64) + np.asarray(P3b, np.float64)
    return out.astype(np.float32)


def kernel(x_sparse, f_sparse, x_dense, W1, b1, W2, b2, W3, b3,
           P1w, P1b, P2w, P2b, P3w, P3b, edge_src, edge_dst):
    gelu1_coefs = _plan(W1, b1)
    in_maps, src, dst = _pack_inputs(x_sparse, f_sparse, x_dense, W1, b1,
                                     W2, b2, W3, b3, edge_src, edge_dst,
                                     gelu1_coefs)
    nc = _build_bass(gelu1_coefs)
    res = run_bass_kernel_spmd(nc, in_maps, list(range(N_CORES)))
    return _host_tail(res.results, src, dst, f_sparse, b3,
                      P1w, P1b, P2w, P2b, P3w, P3b)


def run_profiled(inputs, tmpdir=None):
    """Run once with tracing enabled; returns BassKernelResults."""
    kw = {k: inputs[k] for k in ("x_sparse", "f_sparse", "x_dense", "W1",
                                 "b1", "W2", "b2", "W3", "b3",
                                 "edge_src", "edge_dst")}
    gelu1_coefs = _plan(kw["W1"], kw["b1"])
    in_maps, _, _ = _pack_inputs(**kw, gelu1_coefs=gelu1_coefs)
    nc = _build_bass(gelu1_coefs)
    return run_bass_kernel_spmd(nc, in_maps, list(range(N_CORES)),
                                trace=True, tmpdir=tmpdir)
